# revision 30
# baseline (speedup 1.0000x reference)
"""Multi-head causal self-attention (B=256, T=256, C=384, H=6, D=64) on 8
Trainium2 NeuronCores, data-parallel over the batch dimension (32 batches per
core, no collectives).

Per-core dataflow (bf16 matmul operands, fp32 PSUM accumulation; the
softmax-normalization chain stays fp32/fp32r):
  - Q/K projections produce transposed activations Qt/Kt [e, t] so the
    score matmul can contract head dims on partitions; V stays [t, e].
  - Scores are computed transposed, St[j, i] (keys on partitions), so the
    softmax numerator exp(St + causal_mask) feeds the P@V matmul directly
    with no on-chip transposes.
  - Softmax denominators come from a ones-row matmul over exp(St); the
    per-query reciprocal is replicated across partitions via a DRAM
    round-trip DMA (engines cannot partition-broadcast from SBUF).
  - Head outputs Ot [c, t] are normalized during PSUM evacuation, then the
    output projection contracts c to give y [t, e] with the bias added from
    a partition-replicated copy of bp.
"""

import numpy as np

import concourse.bass as bass
import concourse.tile as tile
from concourse import mybir
from concourse.bass_utils import run_bass_kernel_spmd

P = 128
B, T, C = 256, 256, 384
H, D = 6, 64
NCORES = 8
BL = B // NCORES  # 32 batches per core
G = 4  # batch group for Q/K projection weight reuse
F32 = mybir.dt.float32
F32R = mybir.dt.float32r
BF16 = mybir.dt.bfloat16
MASK_NEG = -60.0


def _split_drain_waits(nc, cap=1):
    """This container's walrus rejects instructions carrying more than one
    sync wait ("Too many sync wait commands"); hoist extras onto no-ops
    inserted before (same engine => executed in order)."""
    n_new = 0
    for f in nc.m.functions:
        for bb in f.blocks:
            il = bb.instructions
            out = []
            changed = False
            for inst in list(il):
                si = getattr(inst, "sync_info", None)
                if si is not None and len(si.on_wait) > cap:
                    waits = list(si.on_wait)
                    extra, keep = waits[:-cap], waits[-cap:]
                    for i in range(0, len(extra), cap):
                        nop = mybir.InstNoOp(
                            name=f"I-waitsplit-{n_new}",
                            sync_info=mybir.SyncInfo(
                                on_wait=extra[i : i + cap], on_update=[]
                            ),
                            bass_nofuse=True,
                            engine=inst.engine,
                        )
                        n_new += 1
                        out.append(nop)
                    si.on_wait = keep
                    changed = True
                out.append(inst)
            if changed:
                il.clear()
                il.extend(out)
    return n_new


def _act_reciprocal(nc, out, in_):
    eng = nc.scalar
    ins = [eng.lower_ap(in_)]
    for arg in (0.0, 1.0, 0.0):  # bias, scale, alpha
        ins.append(mybir.ImmediateValue(dtype=mybir.dt.float32, value=arg))
    return eng.add_instruction(
        mybir.InstActivation(
            name=nc.get_next_instruction_name(),
            func=mybir.ActivationFunctionType.Reciprocal,
            ins=ins,
            outs=[eng.lower_ap(out)],
        )
    )


def build_module(split_waits=True):
    nc = bass.Bass("TRN2", target_bir_lowering=False, debug=False)

    xt_d = nc.dram_tensor("xt", [C, BL, T], BF16, kind="ExternalInput").ap()
    wq_d = nc.dram_tensor("wq", [C, C], BF16, kind="ExternalInput").ap()
    wk_d = nc.dram_tensor("wk", [C, C], BF16, kind="ExternalInput").ap()
    wv_d = nc.dram_tensor("wv", [C, C], BF16, kind="ExternalInput").ap()
    wp_d = nc.dram_tensor("wp", [C, C], BF16, kind="ExternalInput").ap()
    bp_d = nc.dram_tensor("bp", [C], F32R, kind="ExternalInput").ap()
    mask_d = nc.dram_tensor("mask", [P, P], BF16, kind="ExternalInput").ap()
    y_d = nc.dram_tensor("y", [BL, T, C], F32, kind="ExternalOutput").ap()

    with tile.TileContext(nc) as tc:
        with (
            tc.tile_pool(name="consts", bufs=1) as consts,
            tc.tile_pool(name="xg", bufs=2) as xg_pool,
            tc.tile_pool(name="qt", bufs=4) as qt_pool,
            tc.tile_pool(name="kt", bufs=4) as kt_pool,
            tc.tile_pool(name="vsb", bufs=G + 2) as v_pool,
            tc.tile_pool(name="sts", bufs=14) as sts_pool,
            tc.tile_pool(name="ot", bufs=3) as ot_pool,
            tc.tile_pool(name="ysb", bufs=3) as y_pool,
            tc.tile_pool(name="rsb", bufs=4) as r_pool,
            tc.tile_pool(name="psproj", bufs=2, space="PSUM") as ps_proj,
            tc.tile_pool(name="ps384", bufs=2, space="PSUM") as ps_384,
            tc.tile_pool(name="ps2", bufs=4, space="PSUM") as ps_2,
        ):
            # ---- constants ----
            wq_sb = consts.tile([P, 3, C], BF16)
            wk_sb = consts.tile([P, 3, C], BF16)
            wv_sb = consts.tile([P, 3, C], BF16)
            wp_sb = consts.tile([P, 3, C], BF16)
            for w_sb, w_d in ((wq_sb, wq_d), (wk_sb, wk_d), (wv_sb, wv_d), (wp_sb, wp_d)):
                nc.sync.dma_start(w_sb[:], w_d.rearrange("(co ci) e -> ci co e", ci=P))
            # partition-replication is done with rank-1 matmuls (ones ⊗ row):
            # step-0 partition-broadcast DMAs produce garbage on hardware.
            ones_row = consts.tile([1, P], F32)
            nc.vector.memset(ones_row[:], 1.0)
            ones_row_r = consts.tile([1, P], F32R)
            nc.scalar.activation(
                ones_row_r[:], ones_row[:], mybir.ActivationFunctionType.Copy
            )
            bp_row = consts.tile([1, C], F32R)
            nc.sync.dma_start(bp_row[:], bp_d[None, :])
            bp_sb = consts.tile([P, C], F32)
            mask_sb = consts.tile([P, P], BF16)
            nc.sync.dma_start(mask_sb[:], mask_d[:])
            bp_ps = ps_384.tile([P, C], F32, tag="ps384")
            nc.tensor.matmul(bp_ps[:], ones_row_r[0:1, :], bp_row[0:1, :], start=True, stop=True)
            nc.vector.tensor_copy(bp_sb[:], bp_ps[:])

            ones_mat = consts.tile([P, P], BF16)
            nc.vector.memset(ones_mat[:], 1.0)

            xt_r = xt_d.rearrange("(co ci) b t -> ci co b t", ci=P)

            for g in range(BL // G):
                # ---- load x group [128, 3, G, T] ----
                xg = xg_pool.tile([P, 3, G, T], BF16)
                nc.sync.dma_start(xg[:], xt_r[:, :, g * G : (g + 1) * G, :])

                # ---- Q/K projections for the group (weights stationary) ----
                qt2s, kt2s = [], []
                for w_sb, dst_list in ((wq_sb, qt2s), (wk_sb, kt2s)):
                    pool = qt_pool if w_sb is wq_sb else kt_pool
                    tg = "qtb" if w_sb is wq_sb else "ktb"
                    for bp2 in range(G // 2):
                        dst_list.append(
                            pool.tile(
                                [P, 3, 2, T], BF16, tag=tg, name=f"{tg}_{g}_{bp2}"
                            )
                        )
                    for eo in range(3):
                        for bp2 in range(G // 2):
                            ps = ps_proj.tile([P, 512], F32, tag="psproj")
                            rhs = xg[:, :, 2 * bp2 : 2 * bp2 + 2, :].rearrange(
                                "p c b t -> p c (b t)"
                            )
                            for co in range(3):
                                nc.tensor.matmul(
                                    ps[:],
                                    (w_sb[:, co, eo * P : (eo + 1) * P]),
                                    (rhs[:, co, :]),
                                    start=(co == 0),
                                    stop=(co == 2),
                                )
                            dst_ap = dst_list[bp2][:, eo, :, :].rearrange(
                                "p b t -> p (b t)"
                            )
                            nc.vector.tensor_copy(dst_ap, ps[:])

                for lb in range(G):
                    b = g * G + lb
                    qt = qt2s[lb // 2][:, :, lb % 2, :]
                    kt = kt2s[lb // 2][:, :, lb % 2, :]

                    # ---- V projection: V[t, e] (x stationary) ----
                    v_sb = v_pool.tile([P, 2, H, D], BF16)
                    for tt in range(2):
                        ps = ps_384.tile([P, C], F32, tag="ps384")
                        for co in range(3):
                            nc.tensor.matmul(
                                ps[:],
                                (xg[:, co, lb, tt * P : (tt + 1) * P]),
                                (wv_sb[:, co, :]),
                                start=(co == 0),
                                stop=(co == 2),
                            )
                        nc.vector.tensor_copy(
                            v_sb[:, tt, :, :].rearrange("p h d -> p (h d)"), ps[:]
                        )

                    # ---- scores (transposed) + exp + mask, per head pair ----
                    # st_pair [P, h, jt, T] keeps the pair's numerators
                    # contiguous so the denominator matmul runs at N=512.
                    st_pairs = []
                    for hp in range(3):
                        st_pair = sts_pool.tile([P, 2, 2, T], BF16, tag="stp")
                        for hidx in range(2):
                            h = 2 * hp + hidx
                            co, half = h // 2, h % 2
                            st_ps = ps_2.tile([P, 2, T], F32, tag="ps2")
                            for jt in range(2):
                                nc.tensor.matmul(
                                    st_ps[:, jt, :],
                                    (kt[64 * half : 64 * half + 64, co, jt * P : (jt + 1) * P]),
                                    (qt[64 * half : 64 * half + 64, co, :]),
                                    start=True,
                                    stop=True,
                                )
                            # exp the whole tile unmasked (scores are bounded,
                            # so exp never overflows); the causal mask is
                            # applied multiplicatively below.
                            nc.scalar.activation(
                                st_pair[:, hidx, :, :].rearrange("p a b -> p (a b)"),
                                st_ps[:].rearrange("p a b -> p (a b)"),
                                mybir.ActivationFunctionType.Exp,
                            )
                        # (jt0, i<128) and (jt1, i>=128) are the same [128,128]
                        # 0/1 triangle; (jt1, i<128) is fully masked -> memset
                        # 0 on the idle gpsimd engine.  One op per pair.
                        diag_s = st_pair.rearrange(
                            "p h a (b c) -> p h (a b) c", c=P
                        )[:, :, 0::3, :]
                        nc.gpsimd.tensor_mul(
                            diag_s,
                            diag_s,
                            mask_sb[:, None, None, :].to_broadcast((P, 2, 2, P)),
                        )
                        nc.gpsimd.memset(st_pair[:, :, 1, 0:P], 0.0)
                        st_pairs.append(st_pair)

                    # ---- P@V + denominators + normalize, per head pair ----
                    ot = ot_pool.tile([P, 3, T], BF16)
                    for hp in range(3):
                        h0, h1 = 2 * hp, 2 * hp + 1
                        # bf16 matmuls support col tile_position, so the odd
                        # head writes partitions 64:128 directly
                        st_pair = st_pairs[hp]
                        # even head -> partitions 0:64, odd head -> 64:128
                        # (bf16 col tile_position), sharing one free block so
                        # the normalize is a single full-width op.
                        o_ps = ps_2.tile([P, T], F32, tag="ps2")
                        for idx, h in enumerate((h0, h1)):
                            base = 64 * idx
                            for jt in range(2):
                                nc.tensor.matmul(
                                    o_ps[base : base + 64, :],
                                    (v_sb[:, jt, h, :]),
                                    (st_pair[:, idx, jt, :]),
                                    start=(jt == 0),
                                    stop=(jt == 1),
                                    tile_position=(0, base),
                                )
                        # denominators in the same diagonal layout: the even
                        # head's column sums land on partitions 0:64 (ones
                        # columns 0:64), the odd head's on 64:128.
                        rb_ps = ps_2.tile([P, T], F32, tag="ps2")
                        for idx in range(2):
                            base = 64 * idx
                            for jt in range(2):
                                nc.tensor.matmul(
                                    rb_ps[base : base + 64, :],
                                    (ones_mat[:, base : base + 64]),
                                    (st_pair[:, idx, jt, :]),
                                    start=(jt == 0),
                                    stop=(jt == 1),
                                    tile_position=(0, base),
                                )
                        rb = r_pool.tile([P, T], F32, tag="rb")
                        _act_reciprocal(nc, rb[:], rb_ps[:])
                        nc.vector.tensor_mul(ot[:, hp, :], o_ps[:], rb[:])

                    # ---- output projection + bias ----
                    for tt in range(2):
                        ps = ps_384.tile([P, C], F32, tag="ps384")
                        for co in range(3):
                            nc.tensor.matmul(
                                ps[:],
                                (ot[:, co, tt * P : (tt + 1) * P]),
                                (wp_sb[:, co, :]),
                                start=(co == 0),
                                stop=(co == 2),
                            )
                        y_sb = y_pool.tile([P, C], F32)
                        nc.vector.tensor_add(y_sb[:], ps[:], bp_sb[:])
                        nc.sync.dma_start(y_d[b, tt * P : (tt + 1) * P, :], y_sb[:])

    if split_waits:
        _split_drain_waits(nc)
    return nc


_NC = None


def _get_nc():
    global _NC
    if _NC is None:
        _NC = build_module()
    return _NC


def make_mask():
    # [128, 128] 0/1 triangle block: key p visible to query i when p <= i
    import ml_dtypes

    j = np.arange(P)[:, None]
    i = np.arange(P)[None, :]
    return np.where(j <= i, 1.0, 0.0).astype(ml_dtypes.bfloat16)


def prepare_in_maps(x, Wk, Wq, Wv, Wp, bp):
    import ml_dtypes

    bf16 = ml_dtypes.bfloat16
    xt = np.ascontiguousarray(
        np.asarray(x, dtype=np.float32).transpose(2, 0, 1).astype(bf16)
    )
    # 1/sqrt(D) folded into Wq (exact exponent shift in bf16)
    wq = np.ascontiguousarray((np.asarray(Wq, dtype=np.float32).T * 0.125).astype(bf16))
    wk = np.ascontiguousarray(np.asarray(Wk, dtype=np.float32).T.astype(bf16))
    wv = np.ascontiguousarray(np.asarray(Wv, dtype=np.float32).T.astype(bf16))
    wp = np.ascontiguousarray(np.asarray(Wp, dtype=np.float32).T.astype(bf16))
    bp = np.asarray(bp, dtype=np.float32)
    mask = make_mask()
    in_maps = []
    for c in range(NCORES):
        in_maps.append(
            {
                "xt": np.ascontiguousarray(xt[:, c * BL : (c + 1) * BL, :]),
                "wq": wq,
                "wk": wk,
                "wv": wv,
                "wp": wp,
                "bp": bp,
                "mask": mask,
            }
        )
    return in_maps


def kernel(x, Wk, Wq, Wv, Wp, bp):
    nc = _get_nc()
    in_maps = prepare_in_maps(x, Wk, Wq, Wv, Wp, bp)
    res = run_bass_kernel_spmd(nc, in_maps, list(range(NCORES)))
    return np.concatenate([r["y"] for r in res.results], axis=0)


# revision 31
# speedup vs baseline: 1.0228x; 1.0228x over previous
"""Multi-head causal self-attention (B=256, T=256, C=384, H=6, D=64) on 8
Trainium2 NeuronCores, data-parallel over the batch dimension (32 batches per
core, no collectives).

Per-core dataflow (bf16 matmul operands, fp32 PSUM accumulation; the
softmax-normalization chain stays fp32/fp32r):
  - Q/K projections produce transposed activations Qt/Kt [e, t] so the
    score matmul can contract head dims on partitions; V stays [t, e].
  - Scores are computed transposed, St[j, i] (keys on partitions), so the
    softmax numerator exp(St + causal_mask) feeds the P@V matmul directly
    with no on-chip transposes.
  - Softmax denominators come from a ones-row matmul over exp(St); the
    per-query reciprocal is replicated across partitions via a DRAM
    round-trip DMA (engines cannot partition-broadcast from SBUF).
  - Head outputs Ot [c, t] are normalized during PSUM evacuation, then the
    output projection contracts c to give y [t, e] with the bias added from
    a partition-replicated copy of bp.
"""

import numpy as np

import concourse.bass as bass
import concourse.tile as tile
from concourse import mybir
from concourse.bass_utils import run_bass_kernel_spmd

P = 128
B, T, C = 256, 256, 384
H, D = 6, 64
NCORES = 8
BL = B // NCORES  # 32 batches per core
G = 4  # batch group for Q/K projection weight reuse
F32 = mybir.dt.float32
F32R = mybir.dt.float32r
BF16 = mybir.dt.bfloat16
MASK_NEG = -60.0


def _split_drain_waits(nc, cap=1):
    """This container's walrus rejects instructions carrying more than one
    sync wait ("Too many sync wait commands"); hoist extras onto no-ops
    inserted before (same engine => executed in order)."""
    n_new = 0
    for f in nc.m.functions:
        for bb in f.blocks:
            il = bb.instructions
            out = []
            changed = False
            for inst in list(il):
                si = getattr(inst, "sync_info", None)
                if si is not None and len(si.on_wait) > cap:
                    waits = list(si.on_wait)
                    extra, keep = waits[:-cap], waits[-cap:]
                    for i in range(0, len(extra), cap):
                        nop = mybir.InstNoOp(
                            name=f"I-waitsplit-{n_new}",
                            sync_info=mybir.SyncInfo(
                                on_wait=extra[i : i + cap], on_update=[]
                            ),
                            bass_nofuse=True,
                            engine=inst.engine,
                        )
                        n_new += 1
                        out.append(nop)
                    si.on_wait = keep
                    changed = True
                out.append(inst)
            if changed:
                il.clear()
                il.extend(out)
    return n_new


def _act_reciprocal(nc, out, in_):
    eng = nc.scalar
    ins = [eng.lower_ap(in_)]
    for arg in (0.0, 1.0, 0.0):  # bias, scale, alpha
        ins.append(mybir.ImmediateValue(dtype=mybir.dt.float32, value=arg))
    return eng.add_instruction(
        mybir.InstActivation(
            name=nc.get_next_instruction_name(),
            func=mybir.ActivationFunctionType.Reciprocal,
            ins=ins,
            outs=[eng.lower_ap(out)],
        )
    )


def build_module(split_waits=True):
    nc = bass.Bass("TRN2", target_bir_lowering=False, debug=False)

    xt_d = nc.dram_tensor("xt", [C, BL, T], BF16, kind="ExternalInput").ap()
    wq_d = nc.dram_tensor("wq", [C, C], BF16, kind="ExternalInput").ap()
    wk_d = nc.dram_tensor("wk", [C, C], BF16, kind="ExternalInput").ap()
    wv_d = nc.dram_tensor("wv", [C, C], BF16, kind="ExternalInput").ap()
    wp_d = nc.dram_tensor("wp", [C, C], BF16, kind="ExternalInput").ap()
    bp_d = nc.dram_tensor("bp", [C], F32R, kind="ExternalInput").ap()
    mask_d = nc.dram_tensor("mask", [P, P], BF16, kind="ExternalInput").ap()
    y_d = nc.dram_tensor("y", [BL, T, C], F32, kind="ExternalOutput").ap()

    with tile.TileContext(nc) as tc:
        with (
            tc.tile_pool(name="consts", bufs=1) as consts,
            tc.tile_pool(name="xg", bufs=2) as xg_pool,
            tc.tile_pool(name="qt", bufs=4) as qt_pool,
            tc.tile_pool(name="kt", bufs=4) as kt_pool,
            tc.tile_pool(name="vsb", bufs=G + 2) as v_pool,
            tc.tile_pool(name="sts", bufs=14) as sts_pool,
            tc.tile_pool(name="ot", bufs=3) as ot_pool,
            tc.tile_pool(name="ysb", bufs=3) as y_pool,
            tc.tile_pool(name="rsb", bufs=4) as r_pool,
            tc.tile_pool(name="psproj", bufs=2, space="PSUM") as ps_proj,
            tc.tile_pool(name="ps384", bufs=2, space="PSUM") as ps_384,
            tc.tile_pool(name="ps2", bufs=4, space="PSUM") as ps_2,
        ):
            # ---- constants ----
            wq_sb = consts.tile([P, 3, C], BF16)
            wk_sb = consts.tile([P, 3, C], BF16)
            wv_sb = consts.tile([P, 3, C], BF16)
            wp_sb = consts.tile([P, 3, C], BF16)
            for w_sb, w_d in ((wq_sb, wq_d), (wk_sb, wk_d), (wv_sb, wv_d), (wp_sb, wp_d)):
                nc.sync.dma_start(w_sb[:], w_d.rearrange("(co ci) e -> ci co e", ci=P))
            # partition-replication is done with rank-1 matmuls (ones ⊗ row):
            # step-0 partition-broadcast DMAs produce garbage on hardware.
            ones_row = consts.tile([1, P], F32)
            nc.vector.memset(ones_row[:], 1.0)
            ones_row_r = consts.tile([1, P], F32R)
            nc.scalar.activation(
                ones_row_r[:], ones_row[:], mybir.ActivationFunctionType.Copy
            )
            bp_row = consts.tile([1, C], F32R)
            nc.sync.dma_start(bp_row[:], bp_d[None, :])
            bp_sb = consts.tile([P, C], F32)
            mask_sb = consts.tile([P, P], BF16)
            nc.sync.dma_start(mask_sb[:], mask_d[:])
            bp_ps = ps_384.tile([P, C], F32, tag="ps384")
            nc.tensor.matmul(bp_ps[:], ones_row_r[0:1, :], bp_row[0:1, :], start=True, stop=True)
            nc.vector.tensor_copy(bp_sb[:], bp_ps[:])

            ones_mat = consts.tile([P, P], BF16)
            nc.vector.memset(ones_mat[:], 1.0)

            xt_r = xt_d.rearrange("(co ci) b t -> ci co b t", ci=P)

            for g in range(BL // G):
                # ---- load x group [128, 3, G, T] ----
                xg = xg_pool.tile([P, 3, G, T], BF16)
                nc.sync.dma_start(xg[:], xt_r[:, :, g * G : (g + 1) * G, :])

                # ---- Q/K projections for the group (weights stationary) ----
                qt2s, kt2s = [], []
                for w_sb, dst_list in ((wq_sb, qt2s), (wk_sb, kt2s)):
                    pool = qt_pool if w_sb is wq_sb else kt_pool
                    tg = "qtb" if w_sb is wq_sb else "ktb"
                    for bp2 in range(G // 2):
                        dst_list.append(
                            pool.tile(
                                [P, 3, 2, T], BF16, tag=tg, name=f"{tg}_{g}_{bp2}"
                            )
                        )
                    for eo in range(3):
                        for bp2 in range(G // 2):
                            ps = ps_proj.tile([P, 512], F32, tag="psproj")
                            rhs = xg[:, :, 2 * bp2 : 2 * bp2 + 2, :].rearrange(
                                "p c b t -> p c (b t)"
                            )
                            for co in range(3):
                                nc.tensor.matmul(
                                    ps[:],
                                    (w_sb[:, co, eo * P : (eo + 1) * P]),
                                    (rhs[:, co, :]),
                                    start=(co == 0),
                                    stop=(co == 2),
                                )
                            dst_ap = dst_list[bp2][:, eo, :, :].rearrange(
                                "p b t -> p (b t)"
                            )
                            nc.vector.tensor_copy(dst_ap, ps[:])

                for lb in range(G):
                    b = g * G + lb
                    qt = qt2s[lb // 2][:, :, lb % 2, :]
                    kt = kt2s[lb // 2][:, :, lb % 2, :]

                    # ---- V projection: V[t, e] (x stationary) ----
                    v_sb = v_pool.tile([P, 2, H, D], BF16)
                    for tt in range(2):
                        ps = ps_384.tile([P, C], F32, tag="ps384")
                        for co in range(3):
                            nc.tensor.matmul(
                                ps[:],
                                (xg[:, co, lb, tt * P : (tt + 1) * P]),
                                (wv_sb[:, co, :]),
                                start=(co == 0),
                                stop=(co == 2),
                            )
                        nc.vector.tensor_copy(
                            v_sb[:, tt, :, :].rearrange("p h d -> p (h d)"), ps[:]
                        )

                    # ---- scores (transposed) + exp + mask, per head pair ----
                    # st_pair [P, h, jt, T] keeps the pair's numerators
                    # contiguous so the denominator matmul runs at N=512.
                    st_pairs = []
                    for hp in range(3):
                        st_pair = sts_pool.tile([P, 2, 2, T], BF16, tag="stp")
                        for hidx in range(2):
                            h = 2 * hp + hidx
                            co, half = h // 2, h % 2
                            st_ps = ps_2.tile([P, 2, T], F32, tag="ps2")
                            for jt in range(2):
                                nc.tensor.matmul(
                                    st_ps[:, jt, :],
                                    (kt[64 * half : 64 * half + 64, co, jt * P : (jt + 1) * P]),
                                    (qt[64 * half : 64 * half + 64, co, :]),
                                    start=True,
                                    stop=True,
                                )
                            # exp the whole tile unmasked (scores are bounded,
                            # so exp never overflows); the causal mask is
                            # applied multiplicatively below.
                            nc.scalar.activation(
                                st_pair[:, hidx, :, :].rearrange("p a b -> p (a b)"),
                                st_ps[:].rearrange("p a b -> p (a b)"),
                                mybir.ActivationFunctionType.Exp,
                            )
                        # (jt0, i<128) and (jt1, i>=128) are the same [128,128]
                        # 0/1 triangle; (jt1, i<128) is fully masked -> memset
                        # 0 on the idle gpsimd engine.  One op per pair.
                        diag_s = st_pair.rearrange(
                            "p h a (b c) -> p h (a b) c", c=P
                        )[:, :, 0::3, :]
                        nc.vector.tensor_mul(
                            diag_s,
                            diag_s,
                            mask_sb[:, None, None, :].to_broadcast((P, 2, 2, P)),
                        )
                        nc.gpsimd.memset(st_pair[:, :, 1, 0:P], 0.0)
                        st_pairs.append(st_pair)

                    # ---- P@V + denominators + normalize, per head pair ----
                    ot = ot_pool.tile([P, 3, T], BF16)
                    for hp in range(3):
                        h0, h1 = 2 * hp, 2 * hp + 1
                        # bf16 matmuls support col tile_position, so the odd
                        # head writes partitions 64:128 directly
                        st_pair = st_pairs[hp]
                        # even head -> partitions 0:64, odd head -> 64:128
                        # (bf16 col tile_position), sharing one free block so
                        # the normalize is a single full-width op.
                        o_ps = ps_2.tile([P, T], F32, tag="ps2")
                        for idx, h in enumerate((h0, h1)):
                            base = 64 * idx
                            for jt in range(2):
                                nc.tensor.matmul(
                                    o_ps[base : base + 64, :],
                                    (v_sb[:, jt, h, :]),
                                    (st_pair[:, idx, jt, :]),
                                    start=(jt == 0),
                                    stop=(jt == 1),
                                    tile_position=(0, base),
                                )
                        # denominators in the same diagonal layout: the even
                        # head's column sums land on partitions 0:64 (ones
                        # columns 0:64), the odd head's on 64:128.
                        rb_ps = ps_2.tile([P, T], F32, tag="ps2")
                        for idx in range(2):
                            base = 64 * idx
                            for jt in range(2):
                                nc.tensor.matmul(
                                    rb_ps[base : base + 64, :],
                                    (ones_mat[:, base : base + 64]),
                                    (st_pair[:, idx, jt, :]),
                                    start=(jt == 0),
                                    stop=(jt == 1),
                                    tile_position=(0, base),
                                )
                        rb = r_pool.tile([P, T], F32, tag="rb")
                        _act_reciprocal(nc, rb[:], rb_ps[:])
                        nc.vector.tensor_mul(ot[:, hp, :], o_ps[:], rb[:])

                    # ---- output projection + bias ----
                    for tt in range(2):
                        ps = ps_384.tile([P, C], F32, tag="ps384")
                        for co in range(3):
                            nc.tensor.matmul(
                                ps[:],
                                (ot[:, co, tt * P : (tt + 1) * P]),
                                (wp_sb[:, co, :]),
                                start=(co == 0),
                                stop=(co == 2),
                            )
                        y_sb = y_pool.tile([P, C], F32)
                        nc.vector.tensor_add(y_sb[:], ps[:], bp_sb[:])
                        nc.sync.dma_start(y_d[b, tt * P : (tt + 1) * P, :], y_sb[:])

    if split_waits:
        _split_drain_waits(nc)
    return nc


_NC = None


def _get_nc():
    global _NC
    if _NC is None:
        _NC = build_module()
    return _NC


def make_mask():
    # [128, 128] 0/1 triangle block: key p visible to query i when p <= i
    import ml_dtypes

    j = np.arange(P)[:, None]
    i = np.arange(P)[None, :]
    return np.where(j <= i, 1.0, 0.0).astype(ml_dtypes.bfloat16)


def prepare_in_maps(x, Wk, Wq, Wv, Wp, bp):
    import ml_dtypes

    bf16 = ml_dtypes.bfloat16
    xt = np.ascontiguousarray(
        np.asarray(x, dtype=np.float32).transpose(2, 0, 1).astype(bf16)
    )
    # 1/sqrt(D) folded into Wq (exact exponent shift in bf16)
    wq = np.ascontiguousarray((np.asarray(Wq, dtype=np.float32).T * 0.125).astype(bf16))
    wk = np.ascontiguousarray(np.asarray(Wk, dtype=np.float32).T.astype(bf16))
    wv = np.ascontiguousarray(np.asarray(Wv, dtype=np.float32).T.astype(bf16))
    wp = np.ascontiguousarray(np.asarray(Wp, dtype=np.float32).T.astype(bf16))
    bp = np.asarray(bp, dtype=np.float32)
    mask = make_mask()
    in_maps = []
    for c in range(NCORES):
        in_maps.append(
            {
                "xt": np.ascontiguousarray(xt[:, c * BL : (c + 1) * BL, :]),
                "wq": wq,
                "wk": wk,
                "wv": wv,
                "wp": wp,
                "bp": bp,
                "mask": mask,
            }
        )
    return in_maps


def kernel(x, Wk, Wq, Wv, Wp, bp):
    nc = _get_nc()
    in_maps = prepare_in_maps(x, Wk, Wq, Wv, Wp, bp)
    res = run_bass_kernel_spmd(nc, in_maps, list(range(NCORES)))
    return np.concatenate([r["y"] for r in res.results], axis=0)
